# revision 1
# baseline (speedup 1.0000x reference)
"""Trainium2 Bass kernel for GCFAgg-style block:
    q1 = x@W1.T+b1; q2 = x@W2.T+b2; r = x@WR.T+br
    out = (q1 @ q2.T) @ r        (per batch, no softmax)

Key algebraic restructuring: with x_aug = [x | 1] and W*_aug = [W* | b*],
    out = x_aug @ (Khat @ (x_aug.T @ x_aug) @ Rhat)
where Khat = W1_aug.T @ W2_aug and Rhat = WR_aug.T are tiny host-precomputed
matrices. The device only computes G = x.T @ x (per batch, symmetric — only
upper blocks are computed, lower blocks come from PE transposes) plus a small
[640]^2-sized chain and the final projection out = x @ P + v. That's
~3.5 GFLOP/core instead of ~41 GFLOP/core for the naive N x N similarity
materialization. The augmented row/col of G (column sums of x) and the
constant v-broadcast are folded in from host-precomputed side inputs.

Numerics: fp32r matmuls (fp32 storage, single-pass reduced-precision PE
multiply) giving ~2e-4 relative error vs the fp32 reference — ~10x tighter
than bf16 at only ~10-15% more device time.

Sharding: batch dim B=8, one batch per NeuronCore (data parallel, 8 cores).

Self-contained: hardcodes shapes from the problem spec
(x: [8, 4096, 512] f32; W*: [512, 512]; b*: [512]).
"""
import os
import sys

sys.path.insert(0, "/opt/trn_rl_repo")

import numpy as np
import ml_dtypes

import concourse.bass as bass
import concourse.mybir as mybir
import concourse.tile as tile
from concourse import bacc
from concourse.bass_utils import run_bass_kernel_spmd
from concourse.masks import make_identity
from concourse.tile_rust import add_dep_helper

B = 8          # batch -> one per core
N = 4096       # tokens per batch
D = 512        # model dim
GP = 640       # augmented dim 513 padded to 5*128
NCHUNK = GP // 128   # 5
NT = N // 128        # 32 row tiles
N_CORES = 8

F32 = mybir.dt.float32
F32R = mybir.dt.float32r
BF16 = mybir.dt.bfloat16

# mode: "f32r" (fp32 storage, single-pass reduced-precision matmul),
#       "bf16" (bf16 storage+matmul), "f32" (full-precision 4-pass matmul)
MODE = os.environ.get("GCF_MODE", "f32r")

_built = {}


def _build(mode):
    if mode in _built:
        return _built[mode]

    # Storage dtype IS the matmul dtype: the BIR verifier requires fp32r
    # matmul inputs to be produced (DMA'd/copied) as fp32r.
    if mode == "bf16":
        big_mm = BF16
    elif mode == "f32":
        big_mm = F32
    else:
        big_mm = F32R
    big_store = big_mm
    chain_mm = F32 if mode == "f32" else F32R

    def mm_ap(ap, dt):
        return ap if ap.dtype == dt else ap.bitcast(dt)

    nc = bacc.Bacc("TRN2", target_bir_lowering=False, debug=False,
                   num_devices=N_CORES)

    xa_d = nc.dram_tensor("xa", (N, D), big_store, kind="ExternalInput")
    xat_d = nc.dram_tensor("xat", (NT, 128, 4, 128), big_store,
                           kind="ExternalInput")
    khatT_d = nc.dram_tensor("khatT", (GP, GP), chain_mm, kind="ExternalInput")
    rhat_d = nc.dram_tensor("rhat", (GP, D), chain_mm, kind="ExternalInput")
    # host-precomputed augmented pieces of G_aug (they only involve column
    # sums of x, cheap on host): rows 512:640, and the [:, 512:640] blocks
    gext_d = nc.dram_tensor("gext", (128, GP), chain_mm, kind="ExternalInput")
    augblk_d = nc.dram_tensor("augblk", (4, 128, GP - D), chain_mm,
                              kind="ExternalInput")
    m1row_d = nc.dram_tensor("m1row", (1, D), chain_mm, kind="ExternalInput")
    out_d = nc.dram_tensor("out", (N, D), F32, kind="ExternalOutput")

    with tile.TileContext(nc) as tc:
        with (
            tc.tile_pool(name="xa", bufs=16) as xa_pool,
            tc.tile_pool(name="xat", bufs=32) as xat_pool,
            tc.tile_pool(name="const", bufs=1) as const_pool,
            tc.tile_pool(name="gsb", bufs=1) as g_pool,
            tc.tile_pool(name="chain", bufs=1) as chain_pool,
            tc.tile_pool(name="outsb", bufs=6) as out_pool,
        ):
            # ---- constants (via the otherwise-idle GpSimd DMA queue so they
            # don't delay the sync-queue xa/xat streams) ----
            khat_sb = [const_pool.tile([128, GP], chain_mm, tag=f"khat{c}",
                                       name=f"khat{c}") for c in range(NCHUNK)]
            rhat_sb = [const_pool.tile([128, D], chain_mm, tag=f"rhat{c}",
                                       name=f"rhat{c}") for c in range(NCHUNK)]
            ident = const_pool.tile([128, 128], F32, tag="ident")
            make_identity(nc, ident[:])
            # dtype for the K=1 v-broadcast matmul: bitcasting f32r->f32 is
            # size-preserving, but bf16 tiles must stay bf16 (memset can emit
            # bf16/f32, just not f32r)
            v_mm_dt = big_mm if big_mm == BF16 else F32
            ones_row = const_pool.tile([1, 128], v_mm_dt, tag="ones_row")
            nc.vector.memset(ones_row[:], 1.0)

            # ---- phase 1: G = x^T @ x over 32 row tiles; G is symmetric so
            # only the upper block-triangle is computed on PE ----
            g_sb = [g_pool.tile([128, GP], chain_mm, tag=f"g{c}", name=f"g{c}")
                    for c in range(NCHUNK)]
            with tc.tile_pool(name="psG", bufs=1, space="PSUM") as psG_pool:
                ps_ga = [psG_pool.tile([128, D - c * 128], F32, tag=f"ga{c}",
                                       name=f"ga{c}") for c in range(4)]
                gate_mms = []
                for t in range(NT):
                    xa_t = xa_pool.tile([128, D], big_store, tag="xa")
                    nc.sync.dma_start(xa_t[:], xa_d.ap()[t * 128:(t + 1) * 128, :])
                    for c in range(4):
                        mm = nc.tensor.matmul(
                            ps_ga[c][:],
                            mm_ap(xa_t[:, c * 128:(c + 1) * 128], big_mm),
                            mm_ap(xa_t[:, c * 128:D], big_mm),
                            start=(t == 0), stop=(t == NT - 1),
                        )
                        if c == 3:
                            gate_mms.append(mm)
                gate_mm16 = gate_mms[16]
                # constants + host-side G_aug pieces: gated behind mid-G so
                # their DMAs don't compete with the xa stream during warmup
                # (they're first needed at chain time)
                const_dmas = []
                const_dmas.append(nc.gpsimd.dma_start(g_sb[4][:], gext_d.ap()[:]))
                for c in range(4):
                    const_dmas.append(
                        nc.gpsimd.dma_start(g_sb[c][:, D:GP], augblk_d.ap()[c]))
                m1row_sb = const_pool.tile([1, D], chain_mm, tag="m1row")
                const_dmas.append(nc.gpsimd.dma_start(m1row_sb[:], m1row_d.ap()[:]))
                for c in range(NCHUNK):
                    const_dmas.append(nc.gpsimd.dma_start(
                        khat_sb[c][:], khatT_d.ap()[c * 128:(c + 1) * 128, :]))
                    const_dmas.append(nc.gpsimd.dma_start(
                        rhat_sb[c][:], rhat_d.ap()[c * 128:(c + 1) * 128, :]))
                for cd in const_dmas:
                    add_dep_helper(cd.ins, gate_mm16.ins,
                                   reason="const loads gated behind G t=16")
                # upper blocks into SBUF
                for c in range(4):
                    nc.vector.tensor_copy(g_sb[c][:, c * 128:D], ps_ga[c][:])
                # lower blocks = transpose of upper (G symmetric)
                for c2 in range(1, 4):
                    for c1 in range(c2):
                        ps_tr = psG_pool.tile([128, 128], F32, tag="tr", bufs=2)
                        nc.tensor.transpose(
                            ps_tr[:],
                            mm_ap(g_sb[c1][:, c2 * 128:(c2 + 1) * 128], F32),
                            ident[:],
                        )
                        nc.vector.tensor_copy(
                            g_sb[c2][:, c1 * 128:(c1 + 1) * 128], ps_tr[:])

            # ---- phase 2: P = Khat @ G @ Rhat  (small chain) ----
            with tc.tile_pool(name="psC", bufs=2, space="PSUM") as psC_pool:
                # M1 rows 512:640 come from host (m1row = sx_aug @ Rhat);
                # device computes chunks 0..3 only
                m1_sb = [chain_pool.tile([128, D], chain_mm, tag=f"m1{c}",
                                         name=f"m1{c}") for c in range(4)]
                for g1 in range(4):
                    ps = psC_pool.tile([128, D], F32, tag="chain", bufs=3)
                    for g2 in range(NCHUNK):
                        nc.tensor.matmul(
                            ps[:],
                            mm_ap(g_sb[g2][:, g1 * 128:(g1 + 1) * 128], chain_mm),
                            mm_ap(rhat_sb[g2][:], chain_mm),
                            start=(g2 == 0), stop=(g2 == NCHUNK - 1),
                        )
                    nc.vector.tensor_copy(m1_sb[g1][:], ps[:])

                p_sb = [chain_pool.tile([128, D], big_store, tag=f"p{c}",
                                        name=f"p{c}") for c in range(NCHUNK)]
                for g1 in range(NCHUNK):
                    ps = psC_pool.tile([128, D], F32, tag="chain", bufs=3)
                    for g2 in range(4):
                        nc.tensor.matmul(
                            ps[:],
                            mm_ap(khat_sb[g2][:, g1 * 128:(g1 + 1) * 128], chain_mm),
                            mm_ap(m1_sb[g2][:], chain_mm),
                            start=(g2 == 0), stop=False,
                        )
                    # g2=4 contribution: only row 512 of K^T/M1 is nonzero
                    nc.tensor.matmul(
                        ps[:],
                        mm_ap(khat_sb[4][0:1, g1 * 128:(g1 + 1) * 128], chain_mm),
                        mm_ap(m1row_sb[0:1, :], chain_mm),
                        start=False, stop=True,
                    )
                    nc.vector.tensor_copy(p_sb[g1][:], ps[:])

            # ---- phase 3: out = x @ P[0:512] + v,  v = P_aug[512, :] ----
            with tc.tile_pool(name="psO", bufs=1, space="PSUM") as psO_pool:
                # v broadcast to 128 partitions via a K=1 fp32 matmul
                ps_v = psO_pool.tile([128, D], F32, tag="v", bufs=1)
                nc.tensor.matmul(
                    ps_v[:], ones_row[0:1, :], mm_ap(p_sb[4][0:1, :], v_mm_dt),
                    start=True, stop=True,
                )
                v_sb = const_pool.tile([128, D], F32, tag="vsb")
                nc.vector.tensor_copy(v_sb[:], ps_v[:])

                for t in range(NT):
                    xat_t = xat_pool.tile([128, 4, 128], big_store, tag="xat")
                    xdma = nc.scalar.dma_start(xat_t[:], xat_d.ap()[t])
                    # full xat residency, bandwidth-shaped: the xa stream alone
                    # needs ~190GB/s of the ~340GB/s during G, so release xat
                    # at only 1 tile per 2 G tiles there; the remainder streams
                    # during the chain window, which otherwise runs at ~60% BW
                    add_dep_helper(xdma.ins, gate_mms[min(NT - 1, 2 * t + 6)].ins,
                                   reason="xat prefetch BW-shaped behind G")
                    ps = psO_pool.tile([128, D], F32, tag="out", bufs=6)
                    for c in range(4):
                        nc.tensor.matmul(
                            ps[:],
                            mm_ap(xat_t[:, c, :], big_mm),
                            mm_ap(p_sb[c][:], big_mm),
                            start=(c == 0), stop=(c == 3),
                        )
                    ot = out_pool.tile([128, D], F32, tag="ot")
                    nc.vector.tensor_add(ot[:], ps[:], v_sb[:])
                    # alternate store triggers across two queues: a single
                    # queue serializes 32 x ~640ns DMA_DIRECT2D triggers
                    eng = nc.gpsimd if t % 2 == 0 else nc.sync
                    eng.dma_start(out_d.ap()[t * 128:(t + 1) * 128, :], ot[:])

    nc.compile()
    _built[mode] = nc
    return nc


def _prep_host(x, Wq1_w, Wq1_b, Wq2_w, Wq2_b, WR_w, WR_b, mode):
    f = np.float32
    W1a = np.concatenate([Wq1_w, Wq1_b[:, None]], axis=1)   # [512, 513]
    W2a = np.concatenate([Wq2_w, Wq2_b[:, None]], axis=1)
    WRa = np.concatenate([WR_w, WR_b[:, None]], axis=1)

    khatT = np.zeros((GP, GP), f)   # Khat^T = W2a^T @ W1a, padded
    khatT[:D + 1, :D + 1] = (
        W2a.T.astype(np.float64) @ W1a.astype(np.float64)
    ).astype(f)
    rhat = np.zeros((GP, D), f)     # Rhat = WRa^T, padded
    rhat[:D + 1, :] = WRa.T

    # augmented pieces of G_aug = xa^T @ xa that only need column sums of x
    sx = x.sum(axis=1, dtype=np.float64).astype(f)       # [B, 512]
    gext = np.zeros((B, 128, GP), f)                     # G_aug rows 512:640
    gext[:, 0, :D] = sx
    gext[:, 0, D] = float(N)
    augblk = np.zeros((B, 4, 128, GP - D), f)            # G_aug[:512, 512:640]
    augblk[:, :, :, 0] = sx.reshape(B, 4, 128)
    # M1 row 512 = sx_aug @ Rhat (fully host-computable)
    sxa = np.concatenate([sx, np.full((B, 1), float(N), f)], axis=1)  # [B, 513]
    m1row = (sxa.astype(np.float64) @ WRa.T.astype(np.float64)).astype(f)[:, None, :]

    # xat[b, t, p, c, j] = x[b, t*128+j, c*128+p] — per-(t) contiguous
    # [128, 4, 128] lhsT blocks of x^T
    xat = np.ascontiguousarray(
        x.transpose(0, 2, 1)                     # [B, 512, 4096]
         .reshape(B, 4, 128, NT, 128)            # [B, c, p, t, j]
         .transpose(0, 3, 2, 1, 4)               # [B, t, p, c, j]
    )
    xa = x

    if mode == "bf16":
        bf = ml_dtypes.bfloat16
        xa = xa.astype(bf)
        xat = xat.astype(bf)
    else:
        xa = np.ascontiguousarray(xa)
    return xa, xat, khatT, rhat, gext, augblk, m1row


def kernel(x, Wq1_w, Wq1_b, Wq2_w, Wq2_b, WR_w, WR_b):
    x = np.asarray(x, dtype=np.float32)
    args = [np.asarray(a, dtype=np.float32)
            for a in (Wq1_w, Wq1_b, Wq2_w, Wq2_b, WR_w, WR_b)]
    xa, xat, khatT, rhat, gext, augblk, m1row = _prep_host(x, *args, MODE)

    nc = _build(MODE)
    in_maps = [
        {"xa": xa[b], "xat": xat[b], "khatT": khatT, "rhat": rhat,
         "gext": gext[b], "augblk": augblk[b], "m1row": m1row[b]}
        for b in range(B)
    ]
    # the axon-tunneled device occasionally starts in a wedged state
    # (NRT_EXEC_UNIT_UNRECOVERABLE) and recovers on the next attempt
    last_err = None
    for attempt in range(3):
        try:
            res = run_bass_kernel_spmd(nc, in_maps, core_ids=list(range(N_CORES)))
            break
        except Exception as e:  # noqa: BLE001
            last_err = e
            import time as _time
            _time.sleep(2.0)
            try:
                import jax
                jax.clear_caches()
            except Exception:
                pass
    else:
        raise last_err
    return np.stack([res.results[b]["out"] for b in range(B)])



# revision 3
# speedup vs baseline: 1.4353x; 1.4353x over previous
"""Trainium2 Bass kernel for GCFAgg-style block:
    q1 = x@W1.T+b1; q2 = x@W2.T+b2; r = x@WR.T+br
    out = (q1 @ q2.T) @ r        (per batch, no softmax)

Algebraic restructuring (no N x N similarity materialization): with
K = W1^T W2, G = x^T x, sx = colsums(x), the output is
    out = x @ P + 1 v^T
    P = K G WR^T + R1,   R1 = (K sx) bR^T + (W1^T b2)(WR sx + n bR)^T  (rank 2)
    v = (G WR^T)^T (W2^T b1) + host-only terms
R1's factors, v's host terms, K and sx are tiny host-side precomputations;
the device computes G, the small 512^2 chain, and the final projection.

Numerics: G via fp8-e4m3 DoubleRow matmuls (2 K-planes/cycle), chain and
final projection in fp16, f32 PSUM accumulation throughout, fp16 output
store upcast on host. Measured ~8.4e-3 max rel err vs the f32 reference.

Sharding: batch dim B=8, one batch per NeuronCore (data parallel, 8 cores).

Self-contained: hardcodes shapes from the problem spec
(x: [8, 4096, 512] f32; W*: [512, 512]; b*: [512]).
"""
import sys

sys.path.insert(0, "/opt/trn_rl_repo")

import numpy as np
import ml_dtypes

import concourse.bass as bass  # noqa: F401  (import keeps bass registered)
import concourse.mybir as mybir
import concourse.tile as tile
from concourse import bacc
from concourse.bass_utils import run_bass_kernel_spmd
from concourse.masks import make_identity
from concourse.tile_rust import add_dep_helper

B = 8          # batch -> one per core
N = 4096       # tokens per batch
D = 512        # model dim
NG = 16        # 256-row DoubleRow accumulation steps for G
NGS = 8        # xg DMA supertiles (2 steps each)
NT = 32        # 128-row tiles for the final projection
NTS = 8        # xat DMA supertiles (4 tiles each)
NOS = 8        # output store supertiles (4 tiles each)
N_CORES = 8

F32 = mybir.dt.float32
F16 = mybir.dt.float16
F8 = mybir.dt.float8e4
DR = mybir.MatmulPerfMode.DoubleRow

_built = {}


def _build():
    if "nc" in _built:
        return _built["nc"]

    nc = bacc.Bacc("TRN2", target_bir_lowering=False, debug=False,
                   num_devices=N_CORES)

    # xg[s, p, j, i, d] = fp8(x[s*512 + j*256 + i*128 + p, d])
    xg_d = nc.dram_tensor("xg", (NGS, 128, 2, 2, D), F8, kind="ExternalInput")
    # xat[s, p, c, j] covers lhsT tiles of x^T for 4 row-tiles per supertile
    xat_d = nc.dram_tensor("xat", (NTS, 128, 16, 128), F16,
                           kind="ExternalInput")
    # krhat[p, c, :]: c<4 rows of K^T = W2^T W1, c>=4 rows of WR^T
    krhat_d = nc.dram_tensor("krhat", (128, 8, D), F16, kind="ExternalInput")
    ucol_d = nc.dram_tensor("ucol", (128, 4), F16, kind="ExternalInput")
    r1uv_d = nc.dram_tensor("r1uv", (2, 2 * D), F16, kind="ExternalInput")
    vhost_d = nc.dram_tensor("vhost", (1, D), F16, kind="ExternalInput")
    # out[s, p, j, d] = out_row(s*512 + j*128 + p)[d]
    out_d = nc.dram_tensor("out", (NOS, 128, 4, D), F16, kind="ExternalOutput")

    with tile.TileContext(nc) as tc:
        with (
            tc.tile_pool(name="xg", bufs=4) as xg_pool,
            tc.tile_pool(name="xat", bufs=8) as xat_pool,
            tc.tile_pool(name="const", bufs=1) as const_pool,
            tc.tile_pool(name="gsb", bufs=1) as g_pool,
            tc.tile_pool(name="chain", bufs=1) as chain_pool,
            tc.tile_pool(name="outsb", bufs=3) as out_pool,
        ):
            ident = const_pool.tile([128, 128], F16, tag="ident")
            make_identity(nc, ident[:])
            ones_row = const_pool.tile([1, 128], F16, tag="ones")
            nc.vector.memset(ones_row[:], 1.0)
            kr_sb = const_pool.tile([128, 8, D], F16, tag="krhat")
            ucol_sb = const_pool.tile([128, 4], F16, tag="ucol")
            r1uv_sb = const_pool.tile([2, 2 * D], F16, tag="r1uv")
            vhost_sb = const_pool.tile([1, D], F16, tag="vhost")

            def khat(c):
                return kr_sb[:, c, :]

            def rhat(c):
                return kr_sb[:, 4 + c, :]

            # ---- PE warmup: ramp the DVFS clock during DMA bring-up ----
            with tc.tile_pool(name="psW", bufs=1, space="PSUM") as psW_pool:
                ps_w = psW_pool.tile([128, 128], F32, tag="warm")
                for _ in range(8):
                    nc.tensor.matmul(ps_w[:], ident[:], ident[:],
                                     start=True, stop=True)
                warm_sink = const_pool.tile([128, 128], F16, tag="wsink")
                nc.vector.tensor_copy(warm_sink[:], ps_w[:])

            # ---- phase 1: G = x^T x, fp8 DoubleRow, upper block-triangle ----
            with tc.tile_pool(name="psG", bufs=1, space="PSUM") as psG_pool:
                ps_ga = [psG_pool.tile([128, D - c * 128], F32, tag=f"ga{c}",
                                       name=f"ga{c}") for c in range(4)]
                gate_mms = []
                for s in range(NGS):
                    xg_t = xg_pool.tile([128, 2, 2, D], F8, tag="xg")
                    nc.sync.dma_start(xg_t[:], xg_d.ap()[s])
                    for j in range(2):
                        t = 2 * s + j
                        for c in range(4):
                            mm = nc.tensor.matmul(
                                ps_ga[c][:],
                                xg_t[:, j, :, c * 128:(c + 1) * 128],
                                xg_t[:, j, :, c * 128:D],
                                start=(t == 0), stop=(t == NG - 1),
                                perf_mode=DR,
                            )
                            if c == 0:
                                gate_mms.append(mm)

                # constant loads gated so the first xg supertiles land first
                cds = [
                    nc.gpsimd.dma_start(kr_sb[:], krhat_d.ap()[:]),
                    nc.gpsimd.dma_start(ucol_sb[:], ucol_d.ap()[:]),
                    nc.gpsimd.dma_start(r1uv_sb[:], r1uv_d.ap()[:]),
                    nc.gpsimd.dma_start(vhost_sb[:], vhost_d.ap()[:]),
                ]
                for cd in cds:
                    add_dep_helper(cd.ins, gate_mms[2].ins,
                                   reason="const loads after xg warmup")

                # G upper blocks -> SBUF fp16; lower blocks via PE transpose
                g_sb = [g_pool.tile([128, D], F16, tag=f"g{c}", name=f"g{c}")
                        for c in range(4)]
                for c in range(4):
                    nc.vector.tensor_copy(g_sb[c][:, c * 128:D], ps_ga[c][:])
                for c2, c1 in [(1, 0), (2, 0), (3, 0), (2, 1), (3, 1), (3, 2)]:
                    ps_tr = psG_pool.tile([128, 128], F16, tag="tr", bufs=2)
                    nc.tensor.transpose(
                        ps_tr[:], g_sb[c1][:, c2 * 128:(c2 + 1) * 128],
                        ident[:])
                    nc.vector.tensor_copy(
                        g_sb[c2][:, c1 * 128:(c1 + 1) * 128], ps_tr[:])

            # ---- phase 2: M1 = G WR^T; P = K M1 + R1; v row ----
            with tc.tile_pool(name="psC", bufs=2, space="PSUM") as psC_pool:
                m1_sb = [chain_pool.tile([128, D], F16, tag=f"m1{c}",
                                         name=f"m1{c}") for c in range(4)]
                for g1 in range(4):
                    ps = psC_pool.tile([128, D], F32, tag="chain", bufs=3)
                    for g2 in range(4):
                        nc.tensor.matmul(
                            ps[:], g_sb[g2][:, g1 * 128:(g1 + 1) * 128],
                            rhat(g2),
                            start=(g2 == 0), stop=(g2 == 3),
                        )
                    nc.vector.tensor_copy(m1_sb[g1][:], ps[:])

                # v_dev^T = (W2^T b1)^T M1
                ps_vr = psC_pool.tile([1, D], F32, tag="vr", bufs=1)
                for g2 in range(4):
                    nc.tensor.matmul(
                        ps_vr[:], ucol_sb[:, g2:g2 + 1], m1_sb[g2][:],
                        start=(g2 == 0), stop=(g2 == 3),
                    )
                vrow_sb = chain_pool.tile([1, D], F16, tag="vrow")
                nc.vector.tensor_copy(vrow_sb[:], ps_vr[:])

                p_sb = [chain_pool.tile([128, D], F16, tag=f"p{c}",
                                        name=f"p{c}") for c in range(4)]
                for g1 in range(4):
                    ps = psC_pool.tile([128, D], F32, tag="chain", bufs=3)
                    for g2 in range(4):
                        nc.tensor.matmul(
                            ps[:], khat(g2)[:, g1 * 128:(g1 + 1) * 128],
                            m1_sb[g2][:],
                            start=(g2 == 0), stop=False,
                        )
                    # rank-2 correction R1 = U V as a K=2 matmul
                    nc.tensor.matmul(
                        ps[:], r1uv_sb[:, g1 * 128:(g1 + 1) * 128],
                        r1uv_sb[:, D:2 * D],
                        start=False, stop=True,
                    )
                    nc.vector.tensor_copy(p_sb[g1][:], ps[:])

                # v broadcast to 128 partitions via K=1 fp16 matmuls
                ps_v = psC_pool.tile([128, D], F32, tag="vb", bufs=1)
                nc.tensor.matmul(ps_v[:], ones_row[0:1, :], vrow_sb[0:1, :],
                                 start=True, stop=False)
                nc.tensor.matmul(ps_v[:], ones_row[0:1, :], vhost_sb[0:1, :],
                                 start=False, stop=True)
                v_sb = const_pool.tile([128, D], F32, tag="vsb")
                nc.vector.tensor_copy(v_sb[:], ps_v[:])

            # ---- phase 3: out = x @ P + v ----
            with tc.tile_pool(name="psO", bufs=1, space="PSUM") as psO_pool:
                for s in range(NOS):
                    ot4 = out_pool.tile([128, 4, D], F16, tag="ot")
                    for j in range(4):
                        t = 4 * s + j
                        if t % 4 == 0:
                            xat_t = xat_pool.tile([128, 16, 128], F16,
                                                  tag="xat")
                            xdma = nc.scalar.dma_start(xat_t[:],
                                                       xat_d.ap()[t // 4])
                            # xat deferred behind the xg/const stream
                            add_dep_helper(
                                xdma.ins,
                                gate_mms[min(NG - 1, t + 8)].ins,
                                reason="xat deferred behind G stream")
                        ps = psO_pool.tile([128, D], F32, tag="out", bufs=6)
                        for c in range(4):
                            nc.tensor.matmul(
                                ps[:], xat_t[:, (t % 4) * 4 + c, :],
                                p_sb[c][:],
                                start=(c == 0), stop=(c == 3),
                            )
                        nc.vector.tensor_add(ot4[:, j, :], ps[:], v_sb[:])
                    eng = nc.gpsimd if s % 2 == 0 else nc.sync
                    eng.dma_start(out_d.ap()[s], ot4[:])

    nc.compile()
    _built["nc"] = nc
    return nc


def _prep_host(x, Wq1_w, Wq1_b, Wq2_w, Wq2_b, WR_w, WR_b):
    f32, f16, f8 = np.float32, np.float16, ml_dtypes.float8_e4m3fn
    f64 = np.float64
    W1, b1 = Wq1_w.astype(f64), Wq1_b.astype(f64)
    W2, b2 = Wq2_w.astype(f64), Wq2_b.astype(f64)
    WR, bR = WR_w.astype(f64), WR_b.astype(f64)

    K = W1.T @ W2                                 # [512, 512]
    u = W2.T @ b1                                 # [512]
    sx = x.sum(axis=1, dtype=f64)                 # [B, 512]

    # xg[b, s, p, j, i, d] = fp8(x[b, s*512 + j*256 + i*128 + p, d])
    x8 = x.astype(f8)
    xg = np.ascontiguousarray(
        x8.reshape(B, NGS, 2, 2, 128, D).transpose(0, 1, 4, 2, 3, 5))
    # xat[b, s, p, cj, j] = x[b, (4s + cj//4)*128 + j, (cj%4)*128 + p]
    xat = np.ascontiguousarray(
        x.transpose(0, 2, 1)                      # [B, 512, 4096]
         .reshape(B, 4, 128, NT, 128)             # [b, c, p, t, j]
         .transpose(0, 3, 2, 1, 4)                # [b, t, p, c, j]
         .reshape(B, NTS, 4, 128, 4, 128)         # [b, s, tj, p, c, j]
         .transpose(0, 1, 3, 2, 4, 5)             # [b, s, p, tj, c, j]
         .reshape(B, NTS, 128, 16, 128)
         .astype(f16))

    krhat = np.ascontiguousarray(
        np.concatenate([K.T.reshape(4, 128, D), WR.T.reshape(4, 128, D)], 0)
          .transpose(1, 0, 2)).astype(f16)         # [128, 8, 512]
    ucol = np.ascontiguousarray(u.reshape(4, 128).T).astype(f16)  # [128, 4]

    r1uv = np.zeros((B, 2, 2 * D), f16)
    vhost = np.zeros((B, 1, D), f16)
    for b in range(B):
        U = np.stack([K @ sx[b], W1.T @ b2], axis=1)             # [512, 2]
        V = np.stack([bR, WR @ sx[b] + float(N) * bR], axis=0)   # [2, 512]
        r1uv[b, :, :D] = U.T.astype(f16)
        r1uv[b, :, D:] = V.astype(f16)
        vhost[b, 0] = ((b1 @ W2 @ sx[b]) * bR + (b1 @ b2) * (WR @ sx[b])
                       + float(N) * (b1 @ b2) * bR).astype(f16)
    return xg, xat, krhat, ucol, r1uv, vhost


def kernel(x, Wq1_w, Wq1_b, Wq2_w, Wq2_b, WR_w, WR_b):
    x = np.asarray(x, dtype=np.float32)
    args = [np.asarray(a, dtype=np.float32)
            for a in (Wq1_w, Wq1_b, Wq2_w, Wq2_b, WR_w, WR_b)]
    xg, xat, krhat, ucol, r1uv, vhost = _prep_host(x, *args)

    nc = _build()
    in_maps = [
        {"xg": xg[b], "xat": xat[b], "krhat": krhat, "ucol": ucol,
         "r1uv": r1uv[b], "vhost": vhost[b]}
        for b in range(B)
    ]
    # the axon-tunneled device occasionally starts in a wedged state
    # (NRT_EXEC_UNIT_UNRECOVERABLE) and recovers on the next attempt
    last_err = None
    for attempt in range(3):
        try:
            res = run_bass_kernel_spmd(nc, in_maps, core_ids=list(range(N_CORES)))
            break
        except Exception as e:  # noqa: BLE001
            last_err = e
            import time as _time
            _time.sleep(2.0)
            try:
                import jax
                jax.clear_caches()
            except Exception:
                pass
    else:
        raise last_err

    out = np.empty((B, N, D), np.float32)
    for b in range(B):
        ob = res.results[b]["out"].astype(np.float32)    # [8, 128, 4, 512]
        out[b] = ob.transpose(0, 2, 1, 3).reshape(N, D)
    return out


# revision 6
# speedup vs baseline: 1.4959x; 1.0422x over previous
"""Trainium2 Bass kernel for GCFAgg-style block:
    q1 = x@W1.T+b1; q2 = x@W2.T+b2; r = x@WR.T+br
    out = (q1 @ q2.T) @ r        (per batch, no softmax)

Algebraic restructuring (no N x N similarity materialization): with
K = W1^T W2, G = x^T x, sx = colsums(x), the output is
    out = x @ P + 1 v^T
    P = K G WR^T + R1,   R1 = (K sx) bR^T + (W1^T b2)(WR sx + n bR)^T  (rank 2)
    v = (G WR^T)^T (W2^T b1) + host-only terms
R1, v's host terms, K and sx are tiny host-side precomputations; the device
computes G, the small 512^2 chain, and the final projection out = x P + v.

Numerics: G via fp8-e4m3 DoubleRow matmuls (2 K-planes per instruction),
chain and final projection in fp16, f32 PSUM accumulation throughout, fp16
output store upcast on host. Measured ~8.4e-3 max rel err vs the f32
reference (tolerance 2e-2).

Schedule notes: PE warmup matmuls ramp the DVFS clock during NEFF bring-up;
constant loads are bandwidth-staggered behind the fp8 x stream; x^T tiles
(fp16) stream during the chain + projection; copies/adds are spread across
the Vector/Pool/Activation engines to keep PSUM evacuation off the PE
critical path; the v computation hides behind the first projection tiles.

Sharding: batch dim B=8, one batch per NeuronCore (data parallel, 8 cores).

Self-contained: hardcodes shapes from the problem spec
(x: [8, 4096, 512] f32; W*: [512, 512]; b*: [512]).
"""
import sys

sys.path.insert(0, "/opt/trn_rl_repo")

import numpy as np
import ml_dtypes

import concourse.bass as bass  # noqa: F401
import concourse.mybir as mybir
import concourse.tile as tile
from concourse import bacc
from concourse.bass_utils import run_bass_kernel_spmd
from concourse.masks import make_identity
from concourse.tile_rust import add_dep_helper

B = 8          # batch -> one per core
N = 4096       # tokens per batch
D = 512        # model dim
NG = 16        # 256-row DoubleRow accumulation steps for G
NGS = 8        # xg DMA supertiles (2 steps each)
NT = 32        # 128-row tiles for the final projection
NTS = 8        # xat DMA supertiles (4 tiles each)
NOS = 16       # output store supertiles (2 tiles each)
N_CORES = 8

F32 = mybir.dt.float32
F16 = mybir.dt.float16
F8 = mybir.dt.float8e4
DR = mybir.MatmulPerfMode.DoubleRow
ACT_COPY = mybir.ActivationFunctionType.Copy

_built = {}


def _build():
    if "nc" in _built:
        return _built["nc"]

    nc = bacc.Bacc("TRN2", target_bir_lowering=False, debug=False,
                   num_devices=N_CORES)

    # xg[s, p, j, i, d] = fp8(x[s*512 + j*256 + i*128 + p, d])
    xg_d = nc.dram_tensor("xg", (NGS, 128, 2, 2, D), F8, kind="ExternalInput")
    # xat[s, p, c, j] covers lhsT tiles of x^T for 4 row-tiles per supertile
    xat_d = nc.dram_tensor("xat", (NTS, 128, 16, 128), F16,
                           kind="ExternalInput")
    # rhat[p, c, :] = WR^T[c*128 + p, :]; khat[p, c, :] = (W1^T W2)^T[c*128+p]
    rhat_d = nc.dram_tensor("rhat", (128, 4, D), F16, kind="ExternalInput")
    khat_d = nc.dram_tensor("khat", (128, 4, D), F16, kind="ExternalInput")
    r1_d = nc.dram_tensor("r1", (128, 4, D), F16, kind="ExternalInput")
    ucol_d = nc.dram_tensor("ucol", (128, 4), F16, kind="ExternalInput")
    vhost_d = nc.dram_tensor("vhost", (1, D), F16, kind="ExternalInput")
    # out[s, p, j, d] = out_row(s*256 + j*128 + p)[d]
    out_d = nc.dram_tensor("out", (NOS, 128, 2, D), F16, kind="ExternalOutput")

    with tile.TileContext(nc) as tc:
        with (
            tc.tile_pool(name="xg", bufs=4) as xg_pool,
            tc.tile_pool(name="xat", bufs=8) as xat_pool,
            tc.tile_pool(name="const", bufs=1) as const_pool,
            tc.tile_pool(name="gsb", bufs=1) as g_pool,
            tc.tile_pool(name="chain", bufs=1) as chain_pool,
            tc.tile_pool(name="outsb", bufs=4) as out_pool,
        ):
            def copy_to(i, out_ap, in_ap):
                # PSUM-capable copy engines: DVE (vector) and Activation
                if i % 2 == 0:
                    nc.vector.tensor_copy(out_ap, in_ap)
                else:
                    nc.scalar.activation(out_ap, in_ap, ACT_COPY)

            ident = const_pool.tile([128, 128], F16, tag="ident")
            make_identity(nc, ident[:])
            ones_row = const_pool.tile([1, 128], F16, tag="ones")
            nc.vector.memset(ones_row[:], 1.0)
            rhat_sb = const_pool.tile([128, 4, D], F16, tag="rhat")
            khat_sb = const_pool.tile([128, 4, D], F16, tag="khat")
            r1_sb = const_pool.tile([128, 4, D], F16, tag="r1")
            ucol_sb = const_pool.tile([128, 4], F16, tag="ucol")
            vhost_sb = const_pool.tile([1, D], F16, tag="vhost")

            # ---- PE warmup: ramp the DVFS clock during DMA bring-up ----
            with tc.tile_pool(name="psW", bufs=1, space="PSUM") as psW_pool:
                ps_w = psW_pool.tile([128, 128], F32, tag="warm")
                for _ in range(24):
                    nc.tensor.matmul(ps_w[:], ident[:], ident[:],
                                     start=True, stop=True)
                warm_sink = const_pool.tile([128, 128], F16, tag="wsink")
                nc.vector.tensor_copy(warm_sink[:], ps_w[:])

            # ---- phase 1: G = x^T x, fp8 DoubleRow, upper block-triangle ----
            with tc.tile_pool(name="psG", bufs=1, space="PSUM") as psG_pool:
                ps_ga = [psG_pool.tile([128, D - c * 128], F32, tag=f"ga{c}",
                                       name=f"ga{c}") for c in range(4)]
                # per-step [128, 2, 512] fp8 slabs; supertile 0 is split so
                # the very first matmul starts half a transfer earlier
                slabs = []
                for s in range(NGS):
                    if s == 0:
                        for j in range(2):
                            xh = xg_pool.tile([128, 1, 2, D], F8, tag="xg0")
                            nc.sync.dma_start(xh[:],
                                              xg_d.ap()[0][:, j:j + 1, :, :])
                            slabs.append(xh[:, 0])
                    else:
                        xg_t = xg_pool.tile([128, 2, 2, D], F8, tag="xg")
                        nc.sync.dma_start(xg_t[:], xg_d.ap()[s])
                        slabs.append(xg_t[:, 0])
                        slabs.append(xg_t[:, 1])
                gate_mms = []
                for t in range(NG):
                    for c in range(4):
                        mm = nc.tensor.matmul(
                            ps_ga[c][:],
                            slabs[t][:, :, c * 128:(c + 1) * 128],
                            slabs[t][:, :, c * 128:D],
                            start=(t == 0), stop=(t == NG - 1),
                            perf_mode=DR,
                        )
                        if c == 0:
                            gate_mms.append(mm)

                # constant loads staggered behind the xg stream
                def gate(dma, idx, why):
                    add_dep_helper(dma.ins, gate_mms[idx].ins, reason=why)

                gate(nc.gpsimd.dma_start(ucol_sb[:], ucol_d.ap()[:]), 2,
                     "small consts early")
                gate(nc.gpsimd.dma_start(vhost_sb[:], vhost_d.ap()[:]), 2,
                     "small consts early")
                gate(nc.gpsimd.dma_start(rhat_sb[:], rhat_d.ap()[:]), 6,
                     "rhat by G end")
                gate(nc.gpsimd.dma_start(khat_sb[:], khat_d.ap()[:]), 10,
                     "khat by M1 end")
                gate(nc.gpsimd.dma_start(r1_sb[:], r1_d.ap()[:]), 12,
                     "r1 by P stage")

                # G upper blocks -> SBUF fp16; lower blocks via PE transpose
                g_sb = [g_pool.tile([128, D], F16, tag=f"g{c}", name=f"g{c}")
                        for c in range(4)]
                for c in range(4):
                    copy_to(c, g_sb[c][:, c * 128:D], ps_ga[c][:])
                for i, (c2, c1) in enumerate(
                        [(1, 0), (2, 0), (3, 0), (2, 1), (3, 1), (3, 2)]):
                    ps_tr = psG_pool.tile([128, 128], F16, tag="tr", bufs=3)
                    nc.tensor.transpose(
                        ps_tr[:], g_sb[c1][:, c2 * 128:(c2 + 1) * 128],
                        ident[:])
                    copy_to(i, g_sb[c2][:, c1 * 128:(c1 + 1) * 128],
                            ps_tr[:])

            # ---- phase 2: M1 = G WR^T; P = K M1 + R1 ----
            with tc.tile_pool(name="psC", bufs=2, space="PSUM") as psC_pool:
                m1_sb = [chain_pool.tile([128, D], F16, tag=f"m1{c}",
                                         name=f"m1{c}") for c in range(4)]
                for g1 in range(4):
                    ps = psC_pool.tile([128, D], F32, tag="chain", bufs=3)
                    for g2 in range(4):
                        nc.tensor.matmul(
                            ps[:], g_sb[g2][:, g1 * 128:(g1 + 1) * 128],
                            rhat_sb[:, g2, :],
                            start=(g2 == 0), stop=(g2 == 3),
                        )
                    copy_to(g1, m1_sb[g1][:], ps[:])

                p_sb = [chain_pool.tile([128, D], F16, tag=f"p{c}",
                                        name=f"p{c}") for c in range(4)]
                for g1 in range(4):
                    ps = psC_pool.tile([128, D], F32, tag="chain", bufs=3)
                    for g2 in range(4):
                        nc.tensor.matmul(
                            ps[:], khat_sb[:, g2, g1 * 128:(g1 + 1) * 128],
                            m1_sb[g2][:],
                            start=(g2 == 0), stop=(g2 == 3),
                        )
                    # fused rank-2 host correction: P = K M1 + R1
                    nc.vector.tensor_add(p_sb[g1][:], ps[:],
                                         r1_sb[:, g1, :])

            # ---- phase 3: out = x @ P + v; the v row/broadcast hides
            # behind the first projection supertile's matmuls ----
            with tc.tile_pool(name="psO", bufs=1, space="PSUM") as psO_pool:
                v_sb = const_pool.tile([128, D], F32, tag="vsb")
                for s in range(NOS):
                    ot2 = out_pool.tile([128, 2, D], F16, tag="ot")
                    pss = []
                    for j in range(2):
                        t = 2 * s + j
                        if t % 4 == 0:
                            xat_t = xat_pool.tile([128, 16, 128], F16,
                                                  tag="xat")
                            xdma = nc.scalar.dma_start(xat_t[:],
                                                       xat_d.ap()[t // 4])
                            add_dep_helper(xdma.ins, gate_mms[NG - 1].ins,
                                           reason="xat after G stream")
                        ps = psO_pool.tile([128, D], F32, tag="out", bufs=5)
                        for c in range(4):
                            nc.tensor.matmul(
                                ps[:], xat_t[:, (t % 4) * 4 + c, :],
                                p_sb[c][:],
                                start=(c == 0), stop=(c == 3),
                            )
                        pss.append(ps)
                    if s == 0:
                        # v row: v_dev^T = (W2^T b1)^T M1, then broadcast.
                        # Emitted behind supertile 0's matmuls so the PE cost
                        # hides before the first adds need v_sb.
                        ps_vr = psO_pool.tile([1, D], F32, tag="vr", bufs=1)
                        for g2 in range(4):
                            nc.tensor.matmul(
                                ps_vr[:], ucol_sb[:, g2:g2 + 1], m1_sb[g2][:],
                                start=(g2 == 0), stop=(g2 == 3),
                            )
                        vrow_sb = chain_pool.tile([1, D], F16, tag="vrow")
                        nc.scalar.activation(vrow_sb[:], ps_vr[:], ACT_COPY)
                        ps_v = psO_pool.tile([128, D], F32, tag="vb", bufs=1)
                        nc.tensor.matmul(ps_v[:], ones_row[0:1, :],
                                         vrow_sb[0:1, :],
                                         start=True, stop=False)
                        nc.tensor.matmul(ps_v[:], ones_row[0:1, :],
                                         vhost_sb[0:1, :],
                                         start=False, stop=True)
                        nc.scalar.activation(v_sb[:], ps_v[:], ACT_COPY)
                    for j in range(2):
                        nc.vector.tensor_add(ot2[:, j, :], pss[j][:], v_sb[:])
                    eng = nc.sync if s % 2 == 0 else nc.scalar
                    eng.dma_start(out_d.ap()[s], ot2[:])

    nc.compile()
    _built["nc"] = nc
    return nc


def _prep_host(x, Wq1_w, Wq1_b, Wq2_w, Wq2_b, WR_w, WR_b):
    f16, f8 = np.float16, ml_dtypes.float8_e4m3fn
    f64 = np.float64
    W1, b1 = Wq1_w.astype(f64), Wq1_b.astype(f64)
    W2, b2 = Wq2_w.astype(f64), Wq2_b.astype(f64)
    WR, bR = WR_w.astype(f64), WR_b.astype(f64)

    K = W1.T @ W2                                 # [512, 512]
    u = W2.T @ b1                                 # [512]
    sx = x.sum(axis=1, dtype=f64)                 # [B, 512]

    # xg[b, s, p, j, i, d] = fp8(x[b, s*512 + j*256 + i*128 + p, d])
    x8 = x.astype(f8)
    xg = np.ascontiguousarray(
        x8.reshape(B, NGS, 2, 2, 128, D).transpose(0, 1, 4, 2, 3, 5))
    xat = np.ascontiguousarray(
        x.transpose(0, 2, 1)                      # [B, 512, 4096]
         .reshape(B, 4, 128, NT, 128)             # [b, c, p, t, j]
         .transpose(0, 3, 2, 1, 4)                # [b, t, p, c, j]
         .reshape(B, NTS, 4, 128, 4, 128)         # [b, s, tj, p, c, j]
         .transpose(0, 1, 3, 2, 4, 5)             # [b, s, p, tj, c, j]
         .reshape(B, NTS, 128, 16, 128)
         .astype(f16))

    def chunked(a):   # [512, 512] -> [128, 4, 512]
        return np.ascontiguousarray(
            a.reshape(4, 128, D).transpose(1, 0, 2)).astype(f16)

    rhat = chunked(WR.T)
    khat = chunked(K.T)
    ucol = np.ascontiguousarray(u.reshape(4, 128).T).astype(f16)  # [128, 4]

    r1 = np.zeros((B, 128, 4, D), f16)
    vhost = np.zeros((B, 1, D), f16)
    for b in range(B):
        U = np.stack([K @ sx[b], W1.T @ b2], axis=1)             # [512, 2]
        V = np.stack([bR, WR @ sx[b] + float(N) * bR], axis=0)   # [2, 512]
        r1[b] = chunked(U @ V)
        vhost[b, 0] = ((b1 @ W2 @ sx[b]) * bR + (b1 @ b2) * (WR @ sx[b])
                       + float(N) * (b1 @ b2) * bR).astype(f16)
    return xg, xat, rhat, khat, r1, ucol, vhost


def kernel(x, Wq1_w, Wq1_b, Wq2_w, Wq2_b, WR_w, WR_b):
    x = np.asarray(x, dtype=np.float32)
    args = [np.asarray(a, dtype=np.float32)
            for a in (Wq1_w, Wq1_b, Wq2_w, Wq2_b, WR_w, WR_b)]
    xg, xat, rhat, khat, r1, ucol, vhost = _prep_host(x, *args)

    nc = _build()
    in_maps = [
        {"xg": xg[b], "xat": xat[b], "rhat": rhat, "khat": khat,
         "r1": r1[b], "ucol": ucol, "vhost": vhost[b]}
        for b in range(B)
    ]
    # the axon-tunneled device occasionally starts in a wedged state
    # (NRT_EXEC_UNIT_UNRECOVERABLE) and recovers on the next attempt
    last_err = None
    for attempt in range(3):
        try:
            res = run_bass_kernel_spmd(nc, in_maps, core_ids=list(range(N_CORES)))
            break
        except Exception as e:  # noqa: BLE001
            last_err = e
            import time as _time
            _time.sleep(2.0)
            try:
                import jax
                jax.clear_caches()
            except Exception:
                pass
    else:
        raise last_err

    out = np.empty((B, N, D), np.float32)
    for b in range(B):
        ob = res.results[b]["out"].astype(np.float32)   # [16, 128, 2, 512]
        out[b] = ob.transpose(0, 2, 1, 3).reshape(N, D)
    return out


# revision 7
# speedup vs baseline: 1.6028x; 1.0714x over previous
"""Trainium2 Bass kernel for GCFAgg-style block:
    q1 = x@W1.T+b1; q2 = x@W2.T+b2; r = x@WR.T+br
    out = (q1 @ q2.T) @ r        (per batch, no softmax)

Algebraic restructuring (no N x N similarity materialization): with
K = W1^T W2, G = x^T x, sx = colsums(x), the output is
    out = x @ P + 1 v^T
    P = K G WR^T + R1,   R1 = (K sx) bR^T + (W1^T b2)(WR sx + n bR)^T  (rank 2)
    v = (G WR^T)^T (W2^T b1) + host-only terms
R1, v's host terms, K and sx are tiny host-side precomputations; the device
computes G, the small 512^2 chain, and the final projection out = x P + v.

Numerics: G via fp8-e4m3 DoubleRow matmuls (2 K-planes per instruction),
chain and final projection in fp16, f32 PSUM accumulation throughout, fp16
output store upcast on host. Measured ~8.4e-3 max rel err vs the f32
reference (tolerance 2e-2).

Schedule notes: PE warmup matmuls ramp the DVFS clock during NEFF bring-up;
constant loads are bandwidth-staggered behind the fp8 x stream; x^T tiles
(fp16) stream during the chain + projection; copies/adds are spread across
the Vector/Pool/Activation engines to keep PSUM evacuation off the PE
critical path; the v computation hides behind the first projection tiles.

Sharding: batch dim B=8, one batch per NeuronCore (data parallel, 8 cores).

Self-contained: hardcodes shapes from the problem spec
(x: [8, 4096, 512] f32; W*: [512, 512]; b*: [512]).
"""
import sys

sys.path.insert(0, "/opt/trn_rl_repo")

import numpy as np
import ml_dtypes

import concourse.bass as bass  # noqa: F401
import concourse.mybir as mybir
import concourse.tile as tile
from concourse import bacc
from concourse.bass_utils import run_bass_kernel_spmd
from concourse.masks import make_identity
from concourse.tile_rust import add_dep_helper

B = 8          # batch -> one per core
N = 4096       # tokens per batch
D = 512        # model dim
NG = 16        # 256-row DoubleRow accumulation steps for G
NGS = 8        # xg DMA supertiles (2 steps each)
NT = 32        # 128-row tiles for the final projection
NTS = 8        # xat DMA supertiles (4 tiles each)
NOS = 16       # output store supertiles (2 tiles each)
N_CORES = 8

F32 = mybir.dt.float32
F16 = mybir.dt.float16
F8 = mybir.dt.float8e4
DR = mybir.MatmulPerfMode.DoubleRow
ACT_COPY = mybir.ActivationFunctionType.Copy

_built = {}


def _build():
    if "nc" in _built:
        return _built["nc"]

    nc = bacc.Bacc("TRN2", target_bir_lowering=False, debug=False,
                   num_devices=N_CORES)

    # xg[s, p, j, i, d] = fp8(x[s*512 + j*256 + i*128 + p, d])
    xg_d = nc.dram_tensor("xg", (NGS, 128, 2, 2, D), F8, kind="ExternalInput")
    # xat[s, p, c, j] covers lhsT tiles of x^T for 4 row-tiles per supertile
    xat_d = nc.dram_tensor("xat", (NTS, 128, 16, 128), F16,
                           kind="ExternalInput")
    # rhat[p, c, :] = WR^T[c*128 + p, :]; khat[p, c, :] = (W1^T W2)^T[c*128+p]
    rhat_d = nc.dram_tensor("rhat", (128, 4, D), F16, kind="ExternalInput")
    khat_d = nc.dram_tensor("khat", (128, 4, D), F16, kind="ExternalInput")
    r1_d = nc.dram_tensor("r1", (128, 4, D), F16, kind="ExternalInput")
    vhost_d = nc.dram_tensor("vhost", (1, D), F16, kind="ExternalInput")
    # out[s, p, j, d] = out_row(s*256 + j*128 + p)[d]
    out_d = nc.dram_tensor("out", (NOS, 128, 2, D), F16, kind="ExternalOutput")

    with tile.TileContext(nc) as tc:
        with (
            tc.tile_pool(name="xg", bufs=6) as xg_pool,
            tc.tile_pool(name="xat", bufs=8) as xat_pool,
            tc.tile_pool(name="const", bufs=1) as const_pool,
            tc.tile_pool(name="gsb", bufs=1) as g_pool,
            tc.tile_pool(name="chain", bufs=1) as chain_pool,
            tc.tile_pool(name="outsb", bufs=4) as out_pool,
        ):
            def copy_to(i, out_ap, in_ap):
                # PSUM-capable copy engines: DVE (vector) and Activation
                if i % 2 == 0:
                    nc.vector.tensor_copy(out_ap, in_ap)
                else:
                    nc.scalar.activation(out_ap, in_ap, ACT_COPY)

            warm_src = const_pool.tile([128, 128], F16, tag="wsrc")
            nc.vector.memset(warm_src[:], 1.0)
            ident = const_pool.tile([128, 128], F16, tag="ident")
            make_identity(nc, ident[:])
            ones_row = const_pool.tile([1, 128], F16, tag="ones")
            nc.vector.memset(ones_row[:], 1.0)
            rhat_sb = const_pool.tile([128, 4, D], F16, tag="rhat")
            khat_sb = const_pool.tile([128, 4, D], F16, tag="khat")
            r1_sb = const_pool.tile([128, 4, D], F16, tag="r1")
            vhost_sb = const_pool.tile([1, D], F16, tag="vhost")

            # ---- PE warmup: ramp the DVFS clock during DMA bring-up ----
            with tc.tile_pool(name="psW", bufs=1, space="PSUM") as psW_pool:
                ps_w = psW_pool.tile([128, 128], F32, tag="warm")
                for _ in range(12):
                    nc.tensor.matmul(ps_w[:], warm_src[:], warm_src[:],
                                     start=True, stop=True)
                warm_sink = const_pool.tile([128, 128], F16, tag="wsink")
                nc.vector.tensor_copy(warm_sink[:], ps_w[:])

            # ---- phase 1: G = x^T x, fp8 DoubleRow, upper block-triangle ----
            with tc.tile_pool(name="psG", bufs=1, space="PSUM") as psG_pool:
                ps_ga = [psG_pool.tile([128, D - c * 128], F32, tag=f"ga{c}",
                                       name=f"ga{c}") for c in range(4)]
                # per-step [128, 2, 512] fp8 slabs; supertile 0 is split so
                # the very first matmul starts half a transfer earlier
                slabs = []
                for s in range(NGS):
                    if s == 0:
                        for j in range(2):
                            xh = xg_pool.tile([128, 1, 2, D], F8, tag="xg0")
                            nc.sync.dma_start(xh[:],
                                              xg_d.ap()[0][:, j:j + 1, :, :])
                            slabs.append(xh[:, 0])
                    else:
                        xg_t = xg_pool.tile([128, 2, 2, D], F8, tag="xg")
                        nc.sync.dma_start(xg_t[:], xg_d.ap()[s])
                        slabs.append(xg_t[:, 0])
                        slabs.append(xg_t[:, 1])
                gate_mms = []
                for t in range(NG):
                    for c in range(4):
                        mm = nc.tensor.matmul(
                            ps_ga[c][:],
                            slabs[t][:, :, c * 128:(c + 1) * 128],
                            slabs[t][:, :, c * 128:D],
                            start=(t == 0), stop=(t == NG - 1),
                            perf_mode=DR,
                        )
                        if c == 0:
                            gate_mms.append(mm)

                # constant loads staggered behind the xg stream
                def gate(dma, idx, why):
                    add_dep_helper(dma.ins, gate_mms[idx].ins, reason=why)

                gate(nc.gpsimd.dma_start(vhost_sb[:], vhost_d.ap()[:]), 0,
                     "small consts early")
                for c, gi in enumerate([0, 2, 4, 6]):
                    gate(nc.gpsimd.dma_start(rhat_sb[:, c:c + 1, :],
                                             rhat_d.ap()[:, c:c + 1, :]), gi,
                         "rhat chunk interleaved with xg")
                for c, gi in enumerate([7, 9, 11, 13]):
                    gate(nc.gpsimd.dma_start(khat_sb[:, c:c + 1, :],
                                             khat_d.ap()[:, c:c + 1, :]), gi,
                         "khat chunk interleaved with xg")
                for c, gi in enumerate([11, 13, 15, 15]):
                    gate(nc.gpsimd.dma_start(r1_sb[:, c:c + 1, :],
                                             r1_d.ap()[:, c:c + 1, :]), gi,
                         "r1 chunk by P stage")

                # G upper blocks -> SBUF fp16; lower blocks via PE transpose
                g_sb = [g_pool.tile([128, D], F16, tag=f"g{c}", name=f"g{c}")
                        for c in range(4)]
                for c in range(4):
                    copy_to(c, g_sb[c][:, c * 128:D], ps_ga[c][:])
                for i, (c2, c1) in enumerate(
                        [(1, 0), (2, 0), (3, 0), (2, 1), (3, 1), (3, 2)]):
                    ps_tr = psG_pool.tile([128, 128], F16, tag="tr", bufs=3)
                    nc.tensor.transpose(
                        ps_tr[:], g_sb[c1][:, c2 * 128:(c2 + 1) * 128],
                        ident[:])
                    copy_to(i, g_sb[c2][:, c1 * 128:(c1 + 1) * 128],
                            ps_tr[:])

            # ---- phase 2: M1 = G WR^T; P = K M1 + R1 ----
            with tc.tile_pool(name="psC", bufs=2, space="PSUM") as psC_pool:
                m1_sb = [chain_pool.tile([128, D], F16, tag=f"m1{c}",
                                         name=f"m1{c}") for c in range(4)]
                for g1 in range(4):
                    ps = psC_pool.tile([128, D], F32, tag="chain", bufs=3)
                    for g2 in range(4):
                        nc.tensor.matmul(
                            ps[:], g_sb[g2][:, g1 * 128:(g1 + 1) * 128],
                            rhat_sb[:, g2, :],
                            start=(g2 == 0), stop=(g2 == 3),
                        )
                    copy_to(g1, m1_sb[g1][:], ps[:])

                p_sb = [chain_pool.tile([128, D], F16, tag=f"p{c}",
                                        name=f"p{c}") for c in range(4)]
                for g1 in range(4):
                    ps = psC_pool.tile([128, D], F32, tag="chain", bufs=3)
                    for g2 in range(4):
                        nc.tensor.matmul(
                            ps[:], khat_sb[:, g2, g1 * 128:(g1 + 1) * 128],
                            m1_sb[g2][:],
                            start=(g2 == 0), stop=(g2 == 3),
                        )
                    # fused rank-2 host correction: P = K M1 + R1
                    nc.vector.tensor_add(p_sb[g1][:], ps[:],
                                         r1_sb[:, g1, :])

            # ---- phase 3: out = x @ P + v; the v row/broadcast hides
            # behind the first projection supertile's matmuls ----
            with tc.tile_pool(name="psO", bufs=1, space="PSUM") as psO_pool:
                v_sb = const_pool.tile([128, D], F32, tag="vsb")
                for s in range(NOS):
                    ot2 = out_pool.tile([128, 2, D], F16, tag="ot")
                    pss = []
                    for j in range(2):
                        t = 2 * s + j
                        if t % 4 == 0:
                            xat_t = xat_pool.tile([128, 16, 128], F16,
                                                  tag="xat")
                            xdma = nc.scalar.dma_start(xat_t[:],
                                                       xat_d.ap()[t // 4])
                            add_dep_helper(xdma.ins, gate_mms[NG - 1].ins,
                                           reason="xat after G stream")
                        ps = psO_pool.tile([128, D], F32, tag="out", bufs=6)
                        for c in range(4):
                            nc.tensor.matmul(
                                ps[:], xat_t[:, (t % 4) * 4 + c, :],
                                p_sb[c][:],
                                start=(c == 0), stop=(c == 3),
                            )
                        pss.append(ps)
                    if s == 0:
                        # v is fully host-computed; broadcast the row to 128
                        # partitions behind supertile 0's matmuls
                        ps_v = psO_pool.tile([128, D], F32, tag="vb", bufs=1)
                        nc.tensor.matmul(ps_v[:], ones_row[0:1, :],
                                         vhost_sb[0:1, :],
                                         start=True, stop=True)
                        nc.scalar.activation(v_sb[:], ps_v[:], ACT_COPY)
                    for j in range(2):
                        nc.vector.tensor_add(ot2[:, j, :], pss[j][:], v_sb[:])
                    eng = nc.sync if s % 2 == 0 else nc.scalar
                    eng.dma_start(out_d.ap()[s], ot2[:])

    nc.compile()
    _built["nc"] = nc
    return nc


def _prep_host(x, Wq1_w, Wq1_b, Wq2_w, Wq2_b, WR_w, WR_b):
    f16, f8 = np.float16, ml_dtypes.float8_e4m3fn
    f64 = np.float64
    W1, b1 = Wq1_w.astype(f64), Wq1_b.astype(f64)
    W2, b2 = Wq2_w.astype(f64), Wq2_b.astype(f64)
    WR, bR = WR_w.astype(f64), WR_b.astype(f64)

    K = W1.T @ W2                                 # [512, 512]
    u = W2.T @ b1                                 # [512]
    sx = x.sum(axis=1, dtype=f64)                 # [B, 512]

    # xg[b, s, p, j, i, d] = fp8(x[b, s*512 + j*256 + i*128 + p, d])
    x8 = x.astype(f8)
    xg = np.ascontiguousarray(
        x8.reshape(B, NGS, 2, 2, 128, D).transpose(0, 1, 4, 2, 3, 5))
    xat = np.ascontiguousarray(
        x.transpose(0, 2, 1)                      # [B, 512, 4096]
         .reshape(B, 4, 128, NT, 128)             # [b, c, p, t, j]
         .transpose(0, 3, 2, 1, 4)                # [b, t, p, c, j]
         .reshape(B, NTS, 4, 128, 4, 128)         # [b, s, tj, p, c, j]
         .transpose(0, 1, 3, 2, 4, 5)             # [b, s, p, tj, c, j]
         .reshape(B, NTS, 128, 16, 128)
         .astype(f16))

    def chunked(a):   # [512, 512] -> [128, 4, 512]
        return np.ascontiguousarray(
            a.reshape(4, 128, D).transpose(1, 0, 2)).astype(f16)

    rhat = chunked(WR.T)
    khat = chunked(K.T)
    r1 = np.zeros((B, 128, 4, D), f16)
    vhost = np.zeros((B, 1, D), f16)
    for b in range(B):
        U = np.stack([K @ sx[b], W1.T @ b2], axis=1)             # [512, 2]
        V = np.stack([bR, WR @ sx[b] + float(N) * bR], axis=0)   # [2, 512]
        r1[b] = chunked(U @ V)
        # v = WR (G u) + host terms;  G u = x^T (x u) is a cheap matvec chain
        xb = x[b].astype(f64)
        gu = xb.T @ (xb @ u)
        vhost[b, 0] = (WR @ gu + (b1 @ W2 @ sx[b]) * bR
                       + (b1 @ b2) * (WR @ sx[b])
                       + float(N) * (b1 @ b2) * bR).astype(f16)
    return xg, xat, rhat, khat, r1, vhost


def kernel(x, Wq1_w, Wq1_b, Wq2_w, Wq2_b, WR_w, WR_b):
    x = np.asarray(x, dtype=np.float32)
    args = [np.asarray(a, dtype=np.float32)
            for a in (Wq1_w, Wq1_b, Wq2_w, Wq2_b, WR_w, WR_b)]
    xg, xat, rhat, khat, r1, vhost = _prep_host(x, *args)

    nc = _build()
    in_maps = [
        {"xg": xg[b], "xat": xat[b], "rhat": rhat, "khat": khat,
         "r1": r1[b], "vhost": vhost[b]}
        for b in range(B)
    ]
    # the axon-tunneled device occasionally starts in a wedged state
    # (NRT_EXEC_UNIT_UNRECOVERABLE) and recovers on the next attempt
    last_err = None
    for attempt in range(3):
        try:
            res = run_bass_kernel_spmd(nc, in_maps, core_ids=list(range(N_CORES)))
            break
        except Exception as e:  # noqa: BLE001
            last_err = e
            import time as _time
            _time.sleep(2.0)
            try:
                import jax
                jax.clear_caches()
            except Exception:
                pass
    else:
        raise last_err

    out = np.empty((B, N, D), np.float32)
    for b in range(B):
        ob = res.results[b]["out"].astype(np.float32)   # [16, 128, 2, 512]
        out[b] = ob.transpose(0, 2, 1, 3).reshape(N, D)
    return out


# revision 8
# speedup vs baseline: 1.7235x; 1.0753x over previous
"""Trainium2 Bass kernel for GCFAgg-style block:
    q1 = x@W1.T+b1; q2 = x@W2.T+b2; r = x@WR.T+br
    out = (q1 @ q2.T) @ r        (per batch, no softmax)

Algebraic restructuring (no N x N similarity materialization): with
K = W1^T W2, G = x^T x, sx = colsums(x), the output is
    out = x @ P + 1 v^T
    P = K G WR^T + R1,   R1 = (K sx) bR^T + (W1^T b2)(WR sx + n bR)^T  (rank 2)
    v = (G WR^T)^T (W2^T b1) + host-only terms
R1, v's host terms, K and sx are tiny host-side precomputations; the device
computes G, the small 512^2 chain, and the final projection out = x P + v.

Numerics: G via fp8-e4m3 DoubleRow matmuls (2 K-planes per instruction),
chain and final projection in fp16, f32 PSUM accumulation throughout, fp16
output store upcast on host. Measured ~8.4e-3 max rel err vs the f32
reference (tolerance 2e-2).

Schedule notes: PE warmup matmuls ramp the DVFS clock during NEFF bring-up;
constant loads are bandwidth-staggered behind the fp8 x stream; x^T tiles
(fp16) stream during the chain + projection; copies/adds are spread across
the Vector/Pool/Activation engines to keep PSUM evacuation off the PE
critical path; the v computation hides behind the first projection tiles.

Sharding: batch dim B=8, one batch per NeuronCore (data parallel, 8 cores).

Self-contained: hardcodes shapes from the problem spec
(x: [8, 4096, 512] f32; W*: [512, 512]; b*: [512]).
"""
import sys

sys.path.insert(0, "/opt/trn_rl_repo")

import numpy as np
import ml_dtypes

import concourse.bass as bass  # noqa: F401
import concourse.mybir as mybir
import concourse.tile as tile
from concourse import bacc
from concourse.bass_utils import run_bass_kernel_spmd
from concourse.masks import make_identity
from concourse.tile_rust import add_dep_helper

B = 8          # batch -> one per core
N = 4096       # tokens per batch
D = 512        # model dim
NG = 16        # 256-row DoubleRow accumulation steps for G
NGS = 8        # xg DMA supertiles (2 steps each)
NT = 32        # 128-row tiles for the final projection
NTS = 8        # xat DMA supertiles (4 tiles each)
NOS = 16       # output store supertiles (2 tiles each)
N_CORES = 8

F32 = mybir.dt.float32
F16 = mybir.dt.float16
F8 = mybir.dt.float8e4
DR = mybir.MatmulPerfMode.DoubleRow
ACT_COPY = mybir.ActivationFunctionType.Copy

_built = {}


def _build():
    if "nc" in _built:
        return _built["nc"]

    nc = bacc.Bacc("TRN2", target_bir_lowering=False, debug=False,
                   num_devices=N_CORES)

    # xg[s, p, j, i, d] = fp8(x[s*512 + j*256 + i*128 + p, d])
    xg_d = nc.dram_tensor("xg", (NGS, 128, 2, 2, D), F8, kind="ExternalInput")
    # xat[s, p, c, j] covers lhsT tiles of x^T for 4 row-tiles per supertile
    xat_d = nc.dram_tensor("xat", (NTS, 128, 16, 128), F16,
                           kind="ExternalInput")
    # rhat[p, c, :] = WR^T[c*128 + p, :]; khat[p, c, :] = (W1^T W2)^T[c*128+p]
    rhat_d = nc.dram_tensor("rhat", (128, 4, D), F16, kind="ExternalInput")
    khat_d = nc.dram_tensor("khat", (128, 4, D), F16, kind="ExternalInput")
    r1_d = nc.dram_tensor("r1", (128, 4, D), F16, kind="ExternalInput")
    vhost_d = nc.dram_tensor("vhost", (1, D), F16, kind="ExternalInput")
    # out[s, p, j, d] = out_row(s*256 + j*128 + p)[d]
    out_d = nc.dram_tensor("out", (NOS, 128, 2, D), F16, kind="ExternalOutput")

    with tile.TileContext(nc) as tc:
        with (
            tc.tile_pool(name="xg", bufs=6) as xg_pool,
            tc.tile_pool(name="xat", bufs=8) as xat_pool,
            tc.tile_pool(name="const", bufs=1) as const_pool,
            tc.tile_pool(name="gsb", bufs=1) as g_pool,
            tc.tile_pool(name="chain", bufs=1) as chain_pool,
            tc.tile_pool(name="outsb", bufs=4) as out_pool,
        ):
            def copy_to(i, out_ap, in_ap):
                # PSUM-capable copy engines: DVE (vector) and Activation
                if i % 2 == 0:
                    nc.vector.tensor_copy(out_ap, in_ap)
                else:
                    nc.scalar.activation(out_ap, in_ap, ACT_COPY)

            warm_src = const_pool.tile([128, 128], F16, tag="wsrc")
            nc.vector.memset(warm_src[:], 1.0)
            ident = const_pool.tile([128, 128], F16, tag="ident")
            make_identity(nc, ident[:])
            ones_row = const_pool.tile([1, 128], F16, tag="ones")
            nc.vector.memset(ones_row[:], 1.0)
            rhat_sb = const_pool.tile([128, 4, D], F16, tag="rhat")
            khat_sb = const_pool.tile([128, 4, D], F16, tag="khat")
            r1_sb = const_pool.tile([128, 4, D], F16, tag="r1")
            vhost_sb = const_pool.tile([1, D], F16, tag="vhost")

            # ---- PE warmup: ramp the DVFS clock during DMA bring-up ----
            with tc.tile_pool(name="psW", bufs=1, space="PSUM") as psW_pool:
                ps_w = psW_pool.tile([128, 128], F32, tag="warm")
                for _ in range(17):
                    nc.tensor.matmul(ps_w[:], warm_src[:], warm_src[:],
                                     start=True, stop=True)
                warm_sink = const_pool.tile([128, 128], F16, tag="wsink")
                nc.vector.tensor_copy(warm_sink[:], ps_w[:])

            # ---- phase 1: G = x^T x, fp8 DoubleRow, upper block-triangle ----
            with tc.tile_pool(name="psG", bufs=1, space="PSUM") as psG_pool:
                ps_ga = [psG_pool.tile([128, D - c * 128], F32, tag=f"ga{c}",
                                       name=f"ga{c}") for c in range(4)]
                # per-step [128, 2, 512] fp8 slabs; supertile 0 is split so
                # the very first matmul starts half a transfer earlier
                slabs = []
                for s in range(NGS):
                    if s == 0:
                        for j in range(2):
                            xh = xg_pool.tile([128, 1, 2, D], F8, tag="xg0")
                            nc.sync.dma_start(xh[:],
                                              xg_d.ap()[0][:, j:j + 1, :, :])
                            slabs.append(xh[:, 0])
                    else:
                        xg_t = xg_pool.tile([128, 2, 2, D], F8, tag="xg")
                        nc.sync.dma_start(xg_t[:], xg_d.ap()[s])
                        slabs.append(xg_t[:, 0])
                        slabs.append(xg_t[:, 1])
                gate_mms = []
                for t in range(NG):
                    for c in range(4):
                        mm = nc.tensor.matmul(
                            ps_ga[c][:],
                            slabs[t][:, :, c * 128:(c + 1) * 128],
                            slabs[t][:, :, c * 128:D],
                            start=(t == 0), stop=(t == NG - 1),
                            perf_mode=DR,
                        )
                        if c == 0:
                            gate_mms.append(mm)

                # constant loads staggered behind the xg stream
                def gate(dma, idx, why):
                    add_dep_helper(dma.ins, gate_mms[idx].ins, reason=why)

                gate(nc.gpsimd.dma_start(vhost_sb[:], vhost_d.ap()[:]), 0,
                     "small consts early")
                for c, gi in enumerate([0, 2, 4, 6]):
                    gate(nc.gpsimd.dma_start(rhat_sb[:, c:c + 1, :],
                                             rhat_d.ap()[:, c:c + 1, :]), gi,
                         "rhat chunk interleaved with xg")
                for c, gi in enumerate([9, 11, 13, 15]):
                    gate(nc.gpsimd.dma_start(khat_sb[:, c:c + 1, :],
                                             khat_d.ap()[:, c:c + 1, :]), gi,
                         "khat chunk interleaved with xg")
                for c in range(4):
                    gate(nc.gpsimd.dma_start(r1_sb[:, c:c + 1, :],
                                             r1_d.ap()[:, c:c + 1, :]),
                         NG - 1, "r1 after G stream")

                # G upper blocks -> SBUF fp16; lower blocks via PE
                # transposes emitted just-in-time between the M1 matmul
                # groups, so the PE never idles on a PSUM-evacuation chain.
                # M1 group order [3,2,1,0]: M1[3] needs only upper blocks.
                g_sb = [g_pool.tile([128, D], F16, tag=f"g{c}", name=f"g{c}")
                        for c in range(4)]
                for c in range(4):
                    copy_to(c, g_sb[c][:, c * 128:D], ps_ga[c][:])

                state = {"tr": 0}

                def transpose_block(c2, c1):
                    ps_tr = psG_pool.tile([128, 128], F16, tag="tr", bufs=2)
                    nc.tensor.transpose(
                        ps_tr[:], g_sb[c1][:, c2 * 128:(c2 + 1) * 128],
                        ident[:])
                    copy_to(state["tr"], g_sb[c2][:, c1 * 128:(c1 + 1) * 128],
                            ps_tr[:])
                    state["tr"] += 1

                with tc.tile_pool(name="psC", bufs=2, space="PSUM") \
                        as psC_pool:
                    m1_sb = [chain_pool.tile([128, D], F16, tag=f"m1{c}",
                                             name=f"m1{c}") for c in range(4)]

                    def m1_group(g1, g2_order):
                        ps = psC_pool.tile([128, D], F32, tag="chain", bufs=2)
                        for i, g2 in enumerate(g2_order):
                            nc.tensor.matmul(
                                ps[:], g_sb[g2][:, g1 * 128:(g1 + 1) * 128],
                                rhat_sb[:, g2, :],
                                start=(i == 0), stop=(i == 3),
                            )
                        copy_to(g1, m1_sb[g1][:], ps[:])

                    m1_group(3, [0, 1, 3, 2])
                    transpose_block(3, 2)
                    m1_group(2, [0, 1, 2, 3])
                    transpose_block(2, 1)
                    transpose_block(3, 1)
                    m1_group(1, [0, 1, 2, 3])
                    transpose_block(1, 0)
                    transpose_block(2, 0)
                    transpose_block(3, 0)
                    m1_group(0, [0, 1, 2, 3])

                    # v is fully host-computed: broadcast the row across
                    # partitions; its copy overlaps the P stage
                    ps_v = psC_pool.tile([128, D], F32, tag="chain", bufs=2)
                    nc.tensor.matmul(ps_v[:], ones_row[0:1, :],
                                     vhost_sb[0:1, :], start=True, stop=True)
                    v_sb = const_pool.tile([128, D], F32, tag="vsb")
                    nc.scalar.activation(v_sb[:], ps_v[:], ACT_COPY)

                    p_sb = [chain_pool.tile([128, D], F16, tag=f"p{c}",
                                            name=f"p{c}") for c in range(4)]
                    for g1 in range(4):
                        ps = psC_pool.tile([128, D], F32, tag="chain", bufs=2)
                        for i, g2 in enumerate([3, 2, 1, 0]):
                            nc.tensor.matmul(
                                ps[:], khat_sb[:, g2, g1 * 128:(g1 + 1) * 128],
                                m1_sb[g2][:],
                                start=(i == 0), stop=(i == 3),
                            )
                        # fused rank-2 host correction: P = K M1 + R1
                        nc.vector.tensor_add(p_sb[g1][:], ps[:],
                                             r1_sb[:, g1, :])

            # ---- phase 3: out = x @ P + v; the v row/broadcast hides
            # behind the first projection supertile's matmuls ----
            with tc.tile_pool(name="psO", bufs=1, space="PSUM") as psO_pool:
                for s in range(NOS):
                    ot2 = out_pool.tile([128, 2, D], F16, tag="ot")
                    pss = []
                    for j in range(2):
                        t = 2 * s + j
                        if t % 4 == 0:
                            xat_t = xat_pool.tile([128, 16, 128], F16,
                                                  tag="xat")
                            xdma = nc.scalar.dma_start(xat_t[:],
                                                       xat_d.ap()[t // 4])
                            add_dep_helper(xdma.ins, gate_mms[NG - 1].ins,
                                           reason="xat after G stream")
                        ps = psO_pool.tile([128, D], F32, tag="out", bufs=6)
                        for c in range(4):
                            nc.tensor.matmul(
                                ps[:], xat_t[:, (t % 4) * 4 + c, :],
                                p_sb[c][:],
                                start=(c == 0), stop=(c == 3),
                            )
                        pss.append(ps)
                    for j in range(2):
                        nc.vector.tensor_add(ot2[:, j, :], pss[j][:], v_sb[:])
                    eng = nc.sync if s % 2 == 0 else nc.scalar
                    eng.dma_start(out_d.ap()[s], ot2[:])

    nc.compile()
    _built["nc"] = nc
    return nc


def _prep_host(x, Wq1_w, Wq1_b, Wq2_w, Wq2_b, WR_w, WR_b):
    f16, f8 = np.float16, ml_dtypes.float8_e4m3fn
    f64 = np.float64
    W1, b1 = Wq1_w.astype(f64), Wq1_b.astype(f64)
    W2, b2 = Wq2_w.astype(f64), Wq2_b.astype(f64)
    WR, bR = WR_w.astype(f64), WR_b.astype(f64)

    K = W1.T @ W2                                 # [512, 512]
    u = W2.T @ b1                                 # [512]
    sx = x.sum(axis=1, dtype=f64)                 # [B, 512]

    # xg[b, s, p, j, i, d] = fp8(x[b, s*512 + j*256 + i*128 + p, d])
    x8 = x.astype(f8)
    xg = np.ascontiguousarray(
        x8.reshape(B, NGS, 2, 2, 128, D).transpose(0, 1, 4, 2, 3, 5))
    xat = np.ascontiguousarray(
        x.transpose(0, 2, 1)                      # [B, 512, 4096]
         .reshape(B, 4, 128, NT, 128)             # [b, c, p, t, j]
         .transpose(0, 3, 2, 1, 4)                # [b, t, p, c, j]
         .reshape(B, NTS, 4, 128, 4, 128)         # [b, s, tj, p, c, j]
         .transpose(0, 1, 3, 2, 4, 5)             # [b, s, p, tj, c, j]
         .reshape(B, NTS, 128, 16, 128)
         .astype(f16))

    def chunked(a):   # [512, 512] -> [128, 4, 512]
        return np.ascontiguousarray(
            a.reshape(4, 128, D).transpose(1, 0, 2)).astype(f16)

    rhat = chunked(WR.T)
    khat = chunked(K.T)
    r1 = np.zeros((B, 128, 4, D), f16)
    vhost = np.zeros((B, 1, D), f16)
    for b in range(B):
        U = np.stack([K @ sx[b], W1.T @ b2], axis=1)             # [512, 2]
        V = np.stack([bR, WR @ sx[b] + float(N) * bR], axis=0)   # [2, 512]
        r1[b] = chunked(U @ V)
        # v = WR (G u) + host terms;  G u = x^T (x u) is a cheap matvec chain
        xb = x[b].astype(f64)
        gu = xb.T @ (xb @ u)
        vhost[b, 0] = (WR @ gu + (b1 @ W2 @ sx[b]) * bR
                       + (b1 @ b2) * (WR @ sx[b])
                       + float(N) * (b1 @ b2) * bR).astype(f16)
    return xg, xat, rhat, khat, r1, vhost


def kernel(x, Wq1_w, Wq1_b, Wq2_w, Wq2_b, WR_w, WR_b):
    x = np.asarray(x, dtype=np.float32)
    args = [np.asarray(a, dtype=np.float32)
            for a in (Wq1_w, Wq1_b, Wq2_w, Wq2_b, WR_w, WR_b)]
    xg, xat, rhat, khat, r1, vhost = _prep_host(x, *args)

    nc = _build()
    in_maps = [
        {"xg": xg[b], "xat": xat[b], "rhat": rhat, "khat": khat,
         "r1": r1[b], "vhost": vhost[b]}
        for b in range(B)
    ]
    # the axon-tunneled device occasionally starts in a wedged state
    # (NRT_EXEC_UNIT_UNRECOVERABLE) and recovers on the next attempt
    last_err = None
    for attempt in range(3):
        try:
            res = run_bass_kernel_spmd(nc, in_maps, core_ids=list(range(N_CORES)))
            break
        except Exception as e:  # noqa: BLE001
            last_err = e
            import time as _time
            _time.sleep(2.0)
            try:
                import jax
                jax.clear_caches()
            except Exception:
                pass
    else:
        raise last_err

    out = np.empty((B, N, D), np.float32)
    for b in range(B):
        ob = res.results[b]["out"].astype(np.float32)   # [16, 128, 2, 512]
        out[b] = ob.transpose(0, 2, 1, 3).reshape(N, D)
    return out
